# revision 30
# baseline (speedup 1.0000x reference)
"""Trainium2 Bass kernel for nn_BambaMixerDecoderLayer_84696755077458.

v2 — tensor-parallel mixer + token-parallel epilogue over 8 NeuronCores:
  - P1/P2 (in_proj, conv, SSD scan): column-sharded like vLLM (heads and
    conv channels split with d_inner; B/C streams replicated).
  - The gated-norm input yz is exchanged with ONE AllToAll (bf16): cores
    switch from feature-sharding to token-sharding.  out_proj, residual,
    ln2 and the whole MLP then run token-locally with full (bf16) weights
    streamed from HBM — no AllReduce anywhere.
  - ln1/ln2/norm_w are folded into weight rows on the host.
Everything on-device is feature-major ([feature, token]); host does layout
transforms (transpose / shard / concat) only.

Self-contained: hardcodes all shapes; needs only /opt/trn_rl_repo on sys.path.
"""
import sys
from contextlib import ExitStack

if '/opt/trn_rl_repo' not in sys.path:
    sys.path.insert(0, '/opt/trn_rl_repo')

import numpy as np
import ml_dtypes

# ---------------------------------------------------------------- constants
H = 2048          # hidden
DIN = 4096        # mamba intermediate
DS = 128          # ssm state
DCONV = 4
NH = 64
HD = 64
FF = 8192
EPS = 1e-5
B, L = 2, 2048
T = B * L                         # 4096 tokens
CONV_DIM = DIN + 2 * DS           # 4352
D_IN_PROJ = 2 * DIN + 2 * DS + NH  # 8512

TP = 8
NHr = NH // TP                    # 8 heads / core
DINr = DIN // TP                  # 512
CONVr = DINr + 2 * DS             # 768 conv channels / core
MPROJ = DINr + DINr + 2 * DS + NHr  # 1288 in_proj cols / core
TW = T // TP                      # 512 tokens / core in the epilogue

Q = 128                           # SSD chunk
NCHUNK = T // Q                   # 32
CPS = L // Q                      # chunks per sequence = 16
NT = 512                          # token tile
NEG = -3.0e38

BF16NP = np.dtype(ml_dtypes.bfloat16)


def _f32(x):
    return np.ascontiguousarray(np.asarray(x, dtype=np.float32))


def _bf16(x):
    return np.ascontiguousarray(np.asarray(x).astype(BF16NP))


# ================================================================ host prep
def host_constants():
    ident = np.eye(128, dtype=np.float32)
    i8 = np.eye(8, dtype=np.float32)
    sel8 = np.zeros((8, 8 * 128), np.float32)
    for h in range(8):
        sel8[h, h * 128:(h + 1) * 128] = 1.0
    negselpair = np.zeros((8, 4 * 256), np.float32)
    for p in range(4):
        negselpair[2 * p, p * 256:p * 256 + 128] = -1.0
        negselpair[2 * p + 1, p * 256 + 128:p * 256 + 256] = -1.0
    ones8 = np.ones((8, 128), np.float32)
    ones1 = np.ones((1, 128), np.float32)
    ones128 = np.ones((128, 1), np.float32)
    tri = np.where(np.arange(Q)[:, None] > np.arange(Q)[None, :], NEG, 0.0)
    trimask2 = np.concatenate([tri, tri], axis=1).astype(np.float32)
    return dict(c_ident=ident, c_i8=i8, c_sel8=sel8, c_negselpair=negselpair,
                c_ones8=ones8, c_ones1=ones1, c_ones128=ones128,
                c_trimask2=trimask2)


def shard_core_inputs(inputs, r):
    """Build the per-core input map (all feature-major)."""
    f8 = np.float64
    w_in = np.asarray(inputs['w_in'], f8)
    ln1 = np.asarray(inputs['ln1_w'], f8)
    ln2 = np.asarray(inputs['ln2_w'], f8)
    normw = np.asarray(inputs['norm_w'], f8)

    # fold ln1 into in_proj rows (contraction-dim scaling)
    w_in = w_in * ln1[:, None]
    zs = slice(DINr * r, DINr * (r + 1))
    xs = slice(DIN + DINr * r, DIN + DINr * (r + 1))
    bs = slice(2 * DIN, 2 * DIN + DS)
    cs = slice(2 * DIN + DS, 2 * DIN + 2 * DS)
    dts = slice(2 * DIN + 2 * DS + NHr * r, 2 * DIN + 2 * DS + NHr * (r + 1))
    w_in_r = np.concatenate(
        [w_in[:, zs], w_in[:, xs], w_in[:, bs], w_in[:, cs], w_in[:, dts]],
        axis=1)

    conv_w = _f32(inputs['conv_w'])
    conv_w_r = np.concatenate([conv_w[DINr * r:DINr * (r + 1)], conv_w[DIN:]],
                              axis=0)
    conv_b = _f32(inputs['conv_b'])
    conv_b_r = np.concatenate([conv_b[DINr * r:DINr * (r + 1)], conv_b[DIN:]],
                              axis=0)

    hs = _f32(inputs['hidden_states']).reshape(T, H)

    A_r = _f32(inputs['A_log'])[NHr * r:NHr * (r + 1)]
    dtb_r = _f32(inputs['dt_bias'])[NHr * r:NHr * (r + 1)]
    D_r = _f32(inputs['D_ssm'])[NHr * r:NHr * (r + 1)]

    m = dict(host_constants())
    m['hsT'] = _bf16(hs.T)                                      # [2048, 4096]
    m['hsW'] = np.ascontiguousarray(hs.T[:, TW * r:TW * (r + 1)])  # [2048,512]
    m['w_in'] = _bf16(w_in_r)                                   # [2048, 1288]
    m['dssm_c'] = np.ascontiguousarray(
        np.repeat(D_r, HD).reshape(4, 128).T)                   # [128, 4]
    # conv weights: [128, 6*4] with [p, pt*4+d]
    m['conv_w'] = np.ascontiguousarray(
        conv_w_r.reshape(6, 128, DCONV).transpose(1, 0, 2).reshape(128, 24))
    m['conv_b'] = np.ascontiguousarray(conv_b_r.reshape(6, 128).T)  # [128, 6]
    m['a_col'] = np.ascontiguousarray((-np.exp(A_r))[:, None])   # [8,1]
    m['dtb_col'] = np.ascontiguousarray(dtb_r[:, None])          # [8,1]
    # epilogue weights — full, bf16, norm scales folded into rows
    m['w_out'] = _bf16(np.asarray(inputs['w_out'], f8) * normw[:, None])
    wgu = np.asarray(inputs['w_gate_up'], f8) * ln2[:, None]
    m['w_gate'] = _bf16(wgu[:, :FF])
    m['w_up'] = _bf16(wgu[:, FF:])
    m['w_down'] = _bf16(inputs['w_down'])                        # [8192, 2048]
    return m


# ================================================================ the kernel
def build(world=TP, debug=False):
    import concourse.mybir as mybir
    import concourse.tile as tile
    from concourse import bacc
    from concourse.alu_op_type import AluOpType as Op

    AF = mybir.ActivationFunctionType
    F32 = mybir.dt.float32
    BF16 = mybir.dt.bfloat16

    nc = bacc.Bacc("TRN2", target_bir_lowering=False, debug=False,
                   num_devices=world)

    F32R = mybir.dt.float32r

    def din(name, shape, dt=F32):
        return nc.dram_tensor(name, list(shape), dt, kind="ExternalInput").ap()

    io = {}
    for name, shape, dt in [
            ('hsT', (H, T), BF16),
            ('hsW', (H, TW), F32R),
            ('w_in', (H, MPROJ), BF16),
            ('dssm_c', (128, 4), F32),
            ('conv_w', (128, 24), F32), ('conv_b', (128, 6), F32),
            ('a_col', (8, 1), F32), ('dtb_col', (8, 1), F32),
            ('w_out', (DIN, H), BF16),
            ('w_gate', (H, FF), BF16), ('w_up', (H, FF), BF16),
            ('w_down', (FF, H), BF16),
            ('c_ident', (128, 128), F32), ('c_i8', (8, 8), F32R),
            ('c_sel8', (8, 1024), F32R), ('c_negselpair', (8, 1024), F32R),
            ('c_ones8', (8, 128), F32R), ('c_ones1', (1, 128), F32R),
            ('c_ones128', (128, 1), F32R), ('c_trimask2', (128, 256), F32)]:
        io[name] = din(name, shape, dt)

    io['out1T'] = nc.dram_tensor("out1T", [H, TW], F32,
                                 kind="ExternalOutput").ap()
    io['resid2T'] = nc.dram_tensor("resid2T", [H, TW], F32,
                                   kind="ExternalOutput").ap()

    skind = "ExternalOutput" if debug else "Internal"
    scr = {}
    scr['z'] = nc.dram_tensor("z_s", [DINr, T], BF16, kind=skind).ap()
    scr['dt'] = nc.dram_tensor("dt_s", [8, T], F32R, kind=skind).ap()
    scr['lA'] = nc.dram_tensor("lA_s", [8, T], F32R, kind=skind).ap()
    scr['x'] = nc.dram_tensor("x_s", [DINr, T], F32R, kind=skind).ap()
    scr['b'] = nc.dram_tensor("b_s", [DS, T], F32R, kind=skind).ap()
    scr['c'] = nc.dram_tensor("c_s", [DS, T], F32R, kind=skind).ap()
    scr['a2a_in'] = nc.dram_tensor("a2a_in", [DIN, TW], BF16,
                                   kind=skind).ap()
    scr['a2a_out'] = nc.dram_tensor("a2a_out", [DIN, TW], BF16,
                                    kind="Internal" if not debug
                                    else "ExternalOutput").ap()

    with tile.TileContext(nc) as tc:
        _body(tc, io, scr, world, debug, mybir, tile, Op, AF)

    nc.compile()
    return nc


def _body(tc, io, scr, world, debug, mybir, tile, Op, AF):
    nc = tc.nc
    F32 = mybir.dt.float32
    F32R = mybir.dt.float32r
    BF16 = mybir.dt.bfloat16

    def mm(out, lhsT, rhs, start, stop, skip=False):
        lt = lhsT.bitcast(F32R) if lhsT.dtype == F32 else lhsT
        rt = rhs.bitcast(F32R) if rhs.dtype == F32 else rhs
        nc.tensor.matmul(out, lt, rt, start=start, stop=stop,
                         skip_group_check=skip)

    def rms_scale(out_ap, ssq_ap, inv_n, pool, eps1, Op, AF):
        """out = 1/sqrt(ssq*inv_n + eps) via DVE reciprocal + ACT sqrt."""
        shp = list(ssq_ap.shape)
        t0 = pool.tile(shp, F32, tag="rms_t0", name="rms_t0")
        nc.vector.tensor_scalar(t0[:], ssq_ap, float(inv_n),
                                eps1[:], Op.mult, Op.add)
        t1 = pool.tile(shp, F32, tag="rms_t1", name="rms_t1")
        nc.vector.reciprocal(t1[:], t0[:])
        nc.scalar.activation(out_ap, t1[:], AF.Sqrt)

    with ExitStack() as ES:
        cpool = ES.enter_context(tc.tile_pool(name="consts", bufs=1))

        # -------------------------------------------------------- constants
        C = {}
        for nm, shape, dt in [
                ('c_ident', (128, 128), F32), ('c_i8', (8, 8), F32R),
                ('c_sel8', (8, 1024), F32R), ('c_negselpair', (8, 1024), F32R),
                ('c_ones8', (8, 128), F32R), ('c_ones1', (1, 128), F32R),
                ('c_ones128', (128, 1), F32R), ('c_trimask2', (128, 256), F32),
                ('dssm_c', (128, 4), F32),
                ('conv_w', (128, 24), F32), ('conv_b', (128, 6), F32),
                ('a_col', (8, 1), F32), ('dtb_col', (8, 1), F32)]:
            t = cpool.tile(list(shape), dt, tag=nm)
            nc.sync.dma_start(t[:], io[nm])
            C[nm] = t
        ident, i8 = C['c_ident'], C['c_i8']
        sel8, negselp = C['c_sel8'], C['c_negselpair']
        ones8, ones1, ones128 = C['c_ones8'], C['c_ones1'], C['c_ones128']
        trimask2 = C['c_trimask2']

        eps1 = cpool.tile([1, 1], F32, tag="eps1", name="eps1")
        nc.vector.memset(eps1[:], float(EPS))

        # rms scales for all tokens, live through P1a+P1b
        rows_a_es = ExitStack()
        rows_a = rows_a_es.enter_context(tc.tile_pool(name="rows_a", bufs=1))
        s_all = rows_a.tile([1, T], F32R, tag="s_all", name="s_all")

        # ======================================================== Phase 1a
        # ln1 stats + z + dt   (ln1 already folded into w_in rows)
        with tc.tile_pool(name="p1wa", bufs=1) as p1wa, \
             tc.tile_pool(name="p1a", bufs=2) as p1a, \
             tc.tile_pool(name="p1ps_a", bufs=1, space="PSUM") as p1ps_a, \
             tc.tile_pool(name="p1ps_b", bufs=2, space="PSUM") as p1ps_b:

            WZA = DINr + NHr        # 520 cols: z then dt
            wza = p1wa.tile([128, 16, WZA], BF16, tag="wza", name="wza")
            nc.sync.dma_start(
                wza[:, :, 0:DINr],
                io['w_in'][:, 0:DINr].rearrange("(kt p) m -> p kt m", p=128))
            nc.sync.dma_start(
                wza[:, :, DINr:WZA],
                io['w_in'][:, 2 * DINr + 2 * DS:MPROJ]
                .rearrange("(kt p) m -> p kt m", p=128))

            for nt in range(T // NT):
                tok0 = nt * NT
                hst = p1a.tile([128, 16, NT], BF16, tag="hst", name="hst")
                nc.sync.dma_start(hst[:], io['hsT'][:, tok0:tok0 + NT]
                                  .rearrange("(kt p) n -> p kt n", p=128))
                # ln1 stats: squares on DVE, column-sum on PE
                ssq_ps = p1ps_a.tile([1, NT], F32, tag="ssq", name="ssq")
                for k in range(16):
                    sq = p1a.tile([128, NT], F32R, tag="sq", name="sq")
                    nc.vector.tensor_tensor(sq[:], hst[:, k, :], hst[:, k, :],
                                            Op.mult)
                    mm(ssq_ps[:], ones128[:], sq[:],
                       start=(k == 0), stop=(k == 15))
                rms_scale(s_all[:, tok0:tok0 + NT], ssq_ps[:], 1.0 / H,
                          p1a, eps1, Op, AF)
                sb_ps = p1ps_a.tile([128, NT], F32, tag="sbps", name="sbps")
                mm(sb_ps[:], ones1[:], s_all[:, tok0:tok0 + NT],
                   start=True, stop=True)
                sb = p1a.tile([128, NT], F32, tag="sb", name="sb")
                nc.vector.tensor_copy(sb[:], sb_ps[:])
                # z m-tiles: matmul on RAW hidden, scale on the way out
                for mi in range(4):
                    ps = p1ps_b.tile([128, NT], F32, tag="mt", name="mt")
                    for k in range(16):
                        mm(ps[:], wza[:, k, mi * 128:(mi + 1) * 128],
                           hst[:, k, :], start=(k == 0), stop=(k == 15))
                    zt = p1a.tile([128, NT], BF16, tag="z", name="z")
                    nc.vector.tensor_tensor(zt[:], ps[:], sb[:], Op.mult)
                    nc.sync.dma_start(
                        scr['z'][mi * 128:(mi + 1) * 128, tok0:tok0 + NT],
                        zt[:])
                # dt m-tile (8 wide)
                dtp = p1ps_a.tile([8, NT], F32, tag="mtdt", name="mtdt")
                for k in range(16):
                    mm(dtp[:], wza[:, k, DINr:WZA], hst[:, k, :],
                       start=(k == 0), stop=(k == 15))
                dt_raw = p1a.tile([8, NT], F32, tag="dtraw", name="dtraw")
                nc.vector.tensor_tensor(dt_raw[:], dtp[:], sb[:8, :], Op.mult)
                e8 = p1a.tile([8, NT], F32, tag="e8", name="e8")
                nc.scalar.activation(e8[:], dt_raw[:], AF.Exp,
                                     bias=C['dtb_col'][:], scale=1.0)
                e8p = p1a.tile([8, NT], F32, tag="e8p", name="e8p")
                nc.vector.tensor_scalar_add(e8p[:], e8[:], 1.0)
                dtt = p1a.tile([8, NT], F32, tag="dtt", name="dtt")
                nc.scalar.activation(dtt[:], e8p[:], AF.Ln)
                nc.sync.dma_start(scr['dt'][:, tok0:tok0 + NT],
                                  dtt[:].bitcast(F32R))
                logda = p1a.tile([8, NT], F32, tag="logda", name="logda")
                nc.vector.tensor_scalar_mul(logda[:], dtt[:], C['a_col'][:])
                lat = p1a.tile([8, NT], F32, tag="lat", name="lat")
                for c in range(NT // Q):
                    nc.vector.tensor_tensor_scan(
                        lat[:, c * Q:(c + 1) * Q],
                        ones8[:, :Q].bitcast(F32), logda[:, c * Q:(c + 1) * Q],
                        0.0, Op.mult, Op.add)
                nc.sync.dma_start(scr['lA'][:, tok0:tok0 + NT],
                                  lat[:].bitcast(F32R))

        # ======================================== Phase 1b + 2: conv + SSD
        with tc.tile_pool(name="p1wb", bufs=1) as p1wb, \
             tc.tile_pool(name="p1b", bufs=2) as p1b, \
             tc.tile_pool(name="convp", bufs=2) as convp, \
             tc.tile_pool(name="p2", bufs=3) as p2, \
             tc.tile_pool(name="p2s", bufs=2) as p2s, \
             tc.tile_pool(name="state", bufs=1) as spool, \
             tc.tile_pool(name="p3f", bufs=2) as p3f, \
             tc.tile_pool(name="p1bps_a", bufs=1, space="PSUM") as p1bps_a, \
             tc.tile_pool(name="p1bps_b", bufs=1, space="PSUM") as p1bps_b, \
             tc.tile_pool(name="p2ps1", bufs=1, space="PSUM") as p2ps1, \
             tc.tile_pool(name="p2ps2", bufs=1, space="PSUM") as p2ps2:

            wxbc = p1wb.tile([128, 16, CONVr], BF16, tag="wxbc", name="wxbc")
            nc.sync.dma_start(
                wxbc[:], io['w_in'][:, DINr:DINr + CONVr]
                .rearrange("(kt p) m -> p kt m", p=128))

            S_all = spool.tile([128, NHr * HD], F32R, tag="S_all",
                               name="S_all")
            nc.vector.memset(S_all[:].bitcast(F32), 0.0)

            halo_prev = None
            for nt in range(T // NT):
                tok0 = nt * NT
                seq_start = (tok0 % L) == 0
                hst = p1b.tile([128, 16, NT], BF16, tag="hst", name="hstb",
                               bufs=1)
                nc.sync.dma_start(hst[:], io['hsT'][:, tok0:tok0 + NT]
                                  .rearrange("(kt p) n -> p kt n", p=128))
                sb_ps = p1bps_a.tile([128, NT], F32, tag="sbps", name="sbps")
                mm(sb_ps[:], ones1[:], s_all[:, tok0:tok0 + NT],
                   start=True, stop=True)
                sb = p1b.tile([128, NT], F32, tag="sb", name="sb")
                nc.vector.tensor_copy(sb[:], sb_ps[:])

                halo = [convp.tile([128, NT + 3], F32, tag=f"halo{pt}",
                                   name=f"halo{pt}") for pt in range(6)]
                for pt in range(6):
                    ps = p1bps_b.tile([128, NT], F32, tag="mt", name="mt")
                    for k in range(16):
                        mm(ps[:], wxbc[:, k, pt * 128:(pt + 1) * 128],
                           hst[:, k, :], start=(k == 0), stop=(k == 15))
                    nc.vector.tensor_tensor(halo[pt][:, 3:3 + NT], ps[:],
                                            sb[:], Op.mult)

                for pt in range(6):
                    if seq_start:
                        nc.vector.memset(halo[pt][:, 0:3], 0.0)
                    else:
                        nc.vector.tensor_copy(halo[pt][:, 0:3],
                                              halo_prev[pt][:, NT:NT + 3])
                    acc = convp.tile([128, NT], F32, tag="cacc", name="cacc")
                    nc.vector.tensor_scalar_mul(
                        acc[:], halo[pt][:, 0:NT],
                        C['conv_w'][:, pt * 4:pt * 4 + 1])
                    for d in range(1, 4):
                        nc.vector.scalar_tensor_tensor(
                            acc[:], halo[pt][:, d:d + NT],
                            C['conv_w'][:, pt * 4 + d:pt * 4 + d + 1],
                            acc[:], Op.mult, Op.add)
                    cact = convp.tile([128, NT], F32R, tag="cact",
                                      name="cact")
                    nc.scalar.activation(cact[:], acc[:], AF.Silu,
                                         bias=C['conv_b'][:, pt:pt + 1],
                                         scale=1.0)
                    if pt < 4:
                        nc.sync.dma_start(
                            scr['x'][pt * 128:(pt + 1) * 128,
                                     tok0:tok0 + NT], cact[:])
                    elif pt == 4:
                        nc.sync.dma_start(scr['b'][:, tok0:tok0 + NT],
                                          cact[:])
                    else:
                        nc.sync.dma_start(scr['c'][:, tok0:tok0 + NT],
                                          cact[:])
                halo_prev = halo

                # ---------------- SSD chunks for this token tile ----------
                dtt2 = p2.tile([8, NT], F32R, tag="dtt2", name="dtt2")
                nc.sync.dma_start(dtt2[:], scr['dt'][:, tok0:tok0 + NT])
                lat2 = p2.tile([8, NT], F32R, tag="lat2", name="lat2")
                nc.sync.dma_start(lat2[:], scr['lA'][:, tok0:tok0 + NT])
                y_sb = p3f.tile([128, 4, NT], F32, tag="ysb", name="ysb")
                for cc_ in range(NT // Q):
                    ch = nt * (NT // Q) + cc_
                    t0 = ch * Q
                    xf = p2.tile([128, 4, Q], F32R, tag="xf", name="xf")
                    nc.sync.dma_start(xf[:], scr['x'][:, t0:t0 + Q]
                                      .rearrange("(pt p) n -> p pt n", p=128))
                    bf = p2.tile([128, Q], F32R, tag="bf", name="bf")
                    nc.sync.dma_start(bf[:], scr['b'][:, t0:t0 + Q])
                    cf = p2.tile([128, Q], F32R, tag="cf", name="cf")
                    nc.sync.dma_start(cf[:], scr['c'][:, t0:t0 + Q])

                    lrow = lat2[:, cc_ * Q:(cc_ + 1) * Q]
                    dtrow = dtt2[:, cc_ * Q:(cc_ + 1) * Q]

                    expl = p2s.tile([8, Q], F32R, tag="expl", name="expl")
                    nc.scalar.activation(expl[:], lrow, AF.Exp)
                    ddr0 = p2s.tile([8, Q], F32, tag="ddr0", name="ddr0")
                    nc.vector.tensor_scalar(ddr0[:], lrow, -1.0,
                                            lrow[:, Q - 1:Q].bitcast(F32),
                                            Op.mult, Op.add)
                    dd_rows = p2s.tile([8, Q], F32R, tag="ddrows",
                                       name="ddrows")
                    nc.scalar.activation(dd_rows[:], ddr0[:], AF.Exp)
                    nc.vector.tensor_tensor(dd_rows[:], dd_rows[:], dtrow,
                                            Op.mult)
                    dg = p2s.tile([8, 8], F32R, tag="dg", name="dg")
                    nc.vector.tensor_scalar_mul(dg[:], i8[:],
                                                expl[:, Q - 1:Q].bitcast(F32))

                    misc = p2ps1.tile([128, 512], F32, tag="misc",
                                      name="misc")
                    g_ps = misc[:, 0:128]
                    btm_ps = misc[:, 128:256]
                    ddcol_ps = misc[:, 256:264]
                    decay_ps = misc[:, 264:272]
                    dtcol_ps = misc[:, 272:280]

                    mm(g_ps, bf[:], cf[:], start=True, stop=True)
                    nc.tensor.transpose(btm_ps, bf[:].bitcast(F32), ident[:])
                    mm(ddcol_ps, dd_rows[:], i8[:], start=True, stop=True)
                    mm(decay_ps, ones8[:], dg[:], start=True, stop=True)
                    mm(dtcol_ps, dtrow, i8[:], start=True, stop=True)
                    btm = p2s.tile([128, Q], F32R, tag="btm", name="btm")
                    nc.vector.tensor_copy(btm[:], btm_ps)

                    xtm_ps = p2ps1.tile([128, 512], F32, tag="xtm",
                                        name="xtm")
                    for pt in range(4):
                        nc.tensor.transpose(xtm_ps[:, pt * 128:(pt + 1) * 128],
                                            xf[:, pt, :].bitcast(F32),
                                            ident[:])
                    xtm = p2s.tile([128, NHr * HD], F32R, tag="xtm_sb",
                                   name="xtm_sb")
                    nc.vector.tensor_copy(xtm[:], xtm_ps[:])
                    xw = p2s.tile([128, NHr * HD], F32R, tag="xw", name="xw")
                    for h in range(NHr):
                        nc.vector.tensor_scalar_mul(
                            xw[:, h * HD:(h + 1) * HD],
                            xtm[:, h * HD:(h + 1) * HD], ddcol_ps[:, h:h + 1])

                    y_ps = [p2ps1.tile([64, 512], F32, tag=f"y{half}",
                                       name=f"y{half}") for half in range(2)]
                    for pr in range(4):
                        h0, h1 = 2 * pr, 2 * pr + 1
                        pairps = p2ps2.tile([128, 512], F32, tag="pairps",
                                            name="pairps")
                        dpair = pairps[:, 0:256]
                        d2 = pairps[:, 256:512]
                        for i, h in enumerate((h0, h1)):
                            half = dpair[:, i * 128:(i + 1) * 128]
                            mm(half, sel8[:, h * 128:(h + 1) * 128], lrow,
                               start=True, stop=False)
                            mm(half, lrow,
                               negselp[:, pr * 256 + i * 128:
                                       pr * 256 + (i + 1) * 128],
                               start=False, stop=True)
                        dmask = p2s.tile([128, 256], F32, tag="dmask",
                                         name="dmask")
                        nc.vector.tensor_tensor(dmask[:], dpair, trimask2[:],
                                                Op.add)
                        w0 = p2s.tile([128, 256], F32, tag="w0", name="w0")
                        nc.scalar.activation(w0[:], dmask[:], AF.Exp)
                        mm(d2[:, 0:128], sel8[:, h0 * 128:(h0 + 1) * 128],
                           expl[:], start=True, stop=True)
                        mm(d2[:, 128:256], sel8[:, h1 * 128:(h1 + 1) * 128],
                           expl[:], start=True, stop=True)
                        for i, h in enumerate((h0, h1)):
                            wt = p2s.tile([128, Q], F32R, tag="wt", name="wt")
                            nc.vector.scalar_tensor_tensor(
                                wt[:], w0[:, i * 128:(i + 1) * 128],
                                dtcol_ps[:, h:h + 1], g_ps, Op.mult, Op.mult)
                            ce = p2s.tile([128, Q], F32R, tag="ce", name="ce")
                            nc.vector.tensor_tensor(
                                ce[:], d2[:, i * 128:(i + 1) * 128], cf[:],
                                Op.mult)
                            yp = y_ps[h // 4]
                            ysl = yp[:, (h % 4) * 128:(h % 4 + 1) * 128]
                            mm(ysl, xtm[:, h * HD:(h + 1) * HD], wt[:],
                               start=True, stop=False)
                            mm(ysl, S_all[:, h * HD:(h + 1) * HD], ce[:],
                               start=False, stop=True)

                    tp_ps = p2ps1.tile([128, 512], F32, tag="tp", name="tp")
                    mm(tp_ps[:], btm[:], xw[:], start=True, stop=True)
                    for h in range(NHr):
                        nc.vector.scalar_tensor_tensor(
                            S_all[:, h * HD:(h + 1) * HD],
                            S_all[:, h * HD:(h + 1) * HD],
                            decay_ps[:, h:h + 1],
                            tp_ps[:, h * HD:(h + 1) * HD],
                            Op.mult, Op.add)

                    for pt in range(4):
                        yp = y_ps[pt // 2]
                        base = (pt % 2) * 256
                        ysl0 = y_sb[0:64, pt, cc_ * Q:(cc_ + 1) * Q]
                        ysl1 = y_sb[64:128, pt, cc_ * Q:(cc_ + 1) * Q]
                        nc.vector.scalar_tensor_tensor(
                            ysl0, xf[0:64, pt, :],
                            C['dssm_c'][0:64, pt:pt + 1],
                            yp[0:64, base:base + 128], Op.mult, Op.add)
                        nc.vector.scalar_tensor_tensor(
                            ysl1, xf[64:128, pt, :],
                            C['dssm_c'][64:128, pt:pt + 1],
                            yp[0:64, base + 128:base + 256], Op.mult, Op.add)

                    if (ch + 1) % CPS == 0 and ch + 1 < NCHUNK:
                        nc.vector.memset(S_all[:].bitcast(F32), 0.0)

                # gated product for this token tile -> AllToAll input
                zt = p3f.tile([128, 4, NT], BF16, tag="zt", name="zt")
                nc.sync.dma_start(zt[:], scr['z'][:, tok0:tok0 + NT]
                                  .rearrange("(pt p) n -> p pt n", p=128))
                for pt in range(4):
                    sz = p3f.tile([128, NT], F32, tag="sz", name="sz")
                    nc.scalar.activation(sz[:], zt[:, pt, :], AF.Silu)
                    yz = p3f.tile([128, NT], BF16, tag="yz", name="yz")
                    nc.vector.tensor_tensor(yz[:], y_sb[:, pt, :], sz[:],
                                            Op.mult)
                    nc.sync.dma_start(
                        scr['a2a_in'][nt * NT + pt * 128:
                                      nt * NT + (pt + 1) * 128, :], yz[:])

        # =================================================== AllToAll (bf16)
        if world > 1:
            nc.gpsimd.collective_compute(
                "AllToAll", Op.bypass, replica_groups=[list(range(world))],
                ins=[scr['a2a_in']], outs=[scr['a2a_out']])
        else:
            nc.sync.dma_start(scr['a2a_out'], scr['a2a_in'])
        rows_a_es.close()

        # ================================================ Epilogue (512 tok)
        # out_proj on full-DIN yz + residual + ln2, all token-local.
        resid_es = ExitStack()
        rpool = resid_es.enter_context(tc.tile_pool(name="rpool", bufs=1))
        mtb = rpool.tile([128, 16, TW], BF16, tag="mtb", name="mtb")

        with tc.tile_pool(name="e1", bufs=2) as e1, \
             tc.tile_pool(name="e1w", bufs=2) as e1w, \
             tc.tile_pool(name="yzp", bufs=1) as yzp, \
             tc.tile_pool(name="e1ps_a", bufs=1, space="PSUM") as e1ps_a, \
             tc.tile_pool(name="e1ps_b", bufs=2, space="PSUM") as e1ps_b:
            mt = yzp.tile([128, 16, TW], F32, tag="mt", name="mt")
            yzt = yzp.tile([128, 32, TW], BF16, tag="yzt", name="yzt")
            nc.sync.dma_start(yzt[:], scr['a2a_out']
                              .rearrange("(kt p) n -> p kt n", p=128))
            # rms stats over full DIN (norm_w folded into w_out rows)
            ssq_ps = e1ps_a.tile([1, TW], F32, tag="ssq3", name="ssq3")
            for k in range(32):
                sq = e1.tile([128, TW], F32R, tag="sq3", name="sq3")
                nc.vector.tensor_tensor(sq[:], yzt[:, k, :], yzt[:, k, :],
                                        Op.mult)
                mm(ssq_ps[:], ones128[:], sq[:],
                   start=(k == 0), stop=(k == 31))
            s3 = e1.tile([1, TW], F32R, tag="s3", name="s3")
            rms_scale(s3[:], ssq_ps[:], 1.0 / DIN, e1, eps1, Op, AF)
            s3b_ps = e1ps_a.tile([128, TW], F32, tag="s3bps", name="s3bps")
            mm(s3b_ps[:], ones1[:], s3[:], start=True, stop=True)
            s3b = e1.tile([128, TW], F32, tag="s3b", name="s3b")
            nc.vector.tensor_copy(s3b[:], s3b_ps[:])

            ssq2_ps = e1ps_a.tile([1, TW], F32, tag="ssq2", name="ssq2")
            for mi in range(16):
                wo = e1w.tile([128, 32, 128], BF16, tag="wo", name="wo")
                nc.sync.dma_start(
                    wo[:], io['w_out'][:, mi * 128:(mi + 1) * 128]
                    .rearrange("(kt p) m -> p kt m", p=128))
                ps = e1ps_b.tile([128, TW], F32, tag="op", name="op")
                for k in range(32):
                    mm(ps[:], wo[:, k, :], yzt[:, k, :],
                       start=(k == 0), stop=(k == 31))
                ht = e1.tile([128, TW], F32R, tag="ht", name="ht")
                nc.sync.dma_start(ht[:],
                                  io['hsW'][mi * 128:(mi + 1) * 128, :])
                # resid = out*s3 + hs
                rt = mt[:, mi, :]
                nc.vector.tensor_tensor(rt, ps[:], s3b[:], Op.mult)
                nc.vector.tensor_tensor(rt, rt, ht[:].bitcast(F32), Op.add)
                nc.sync.dma_start(
                    io['resid2T'][mi * 128:(mi + 1) * 128, :], rt)
                sq2 = e1.tile([128, TW], F32R, tag="sq2", name="sq2")
                nc.vector.tensor_tensor(sq2[:], rt, rt, Op.mult)
                mm(ssq2_ps[:], ones128[:], sq2[:],
                   start=(mi == 0), stop=(mi == 15))
            s2 = e1.tile([1, TW], F32R, tag="s2", name="s2")
            rms_scale(s2[:], ssq2_ps[:], 1.0 / H, e1, eps1, Op, AF)
            sb2_ps = e1ps_a.tile([128, TW], F32, tag="sb2ps", name="sb2ps")
            mm(sb2_ps[:], ones1[:], s2[:], start=True, stop=True)
            sb2 = e1.tile([128, TW], F32, tag="sb2", name="sb2")
            nc.vector.tensor_copy(sb2[:], sb2_ps[:])
            for k in range(16):
                nc.vector.tensor_tensor(mtb[:, k, :], mt[:, k, :], sb2[:],
                                        Op.mult)

        # gate_up -> silu*up -> down  (ln2 folded into w_gate/w_up rows)
        act_es = ExitStack()
        apool = act_es.enter_context(
            tc.tile_pool(name="apool", bufs=1, side="right"))
        act = apool.tile([128, 64, TW], BF16, tag="act", name="act")

        with tc.tile_pool(name="e2", bufs=2) as e2, \
             tc.tile_pool(name="e2w", bufs=3) as e2w, \
             tc.tile_pool(name="e2ps", bufs=2, space="PSUM") as e2ps:
            for mi in range(64):
                wg = e2w.tile([128, 16, 128], BF16, tag="wg", name="wg")
                nc.sync.dma_start(
                    wg[:], io['w_gate'][:, mi * 128:(mi + 1) * 128]
                    .rearrange("(kt p) m -> p kt m", p=128))
                wu = e2w.tile([128, 16, 128], BF16, tag="wu", name="wu")
                nc.sync.dma_start(
                    wu[:], io['w_up'][:, mi * 128:(mi + 1) * 128]
                    .rearrange("(kt p) m -> p kt m", p=128))
                gp = e2ps.tile([128, TW], F32, tag="gp", name="gp")
                for k in range(16):
                    mm(gp[:], wg[:, k, :], mtb[:, k, :],
                       start=(k == 0), stop=(k == 15))
                up = e2ps.tile([128, TW], F32, tag="up", name="up")
                for k in range(16):
                    mm(up[:], wu[:, k, :], mtb[:, k, :],
                       start=(k == 0), stop=(k == 15))
                sg = e2.tile([128, TW], F32, tag="sg", name="sg")
                nc.scalar.activation(sg[:], gp[:], AF.Silu)
                nc.vector.tensor_tensor(act[:, mi, :], sg[:], up[:], Op.mult)
        resid_es.close()

        with tc.tile_pool(name="e3", bufs=2) as e3, \
             tc.tile_pool(name="e3w", bufs=2) as e3w, \
             tc.tile_pool(name="e3ps", bufs=2, space="PSUM") as e3ps:
            for mi in range(16):
                wd = e3w.tile([128, 64, 128], BF16, tag="wd", name="wd")
                nc.sync.dma_start(
                    wd[:], io['w_down'][:, mi * 128:(mi + 1) * 128]
                    .rearrange("(kt p) m -> p kt m", p=128))
                ps = e3ps.tile([128, TW], F32, tag="dp", name="dp")
                for k in range(64):
                    mm(ps[:], wd[:, k, :], act[:, k, :],
                       start=(k == 0), stop=(k == 63))
                ot = e3.tile([128, TW], F32, tag="ot", name="ot")
                nc.vector.tensor_copy(ot[:], ps[:])
                nc.sync.dma_start(io['out1T'][mi * 128:(mi + 1) * 128, :],
                                  ot[:])
        act_es.close()


# ================================================================ entry point
def gather_outputs(res):
    out1T = np.concatenate([res.results[r]['out1T'] for r in range(TP)],
                           axis=1)                      # [H, T] feature-major
    out1 = np.ascontiguousarray(out1T.T).reshape(B, L, H)
    resid2T = np.concatenate([res.results[r]['resid2T'] for r in range(TP)],
                             axis=1)
    resid2 = np.ascontiguousarray(resid2T.T).reshape(B, L, H)
    return out1, resid2


def kernel(**inputs):
    from concourse import bass_utils

    nc = build(world=TP, debug=False)
    in_maps = [shard_core_inputs(inputs, r) for r in range(TP)]
    res = bass_utils.run_bass_kernel_spmd(nc, in_maps, core_ids=list(range(TP)))
    return gather_outputs(res)


if __name__ == '__main__':
    nc = build(world=1)
    print("built ok")
